# revision 13
# baseline (speedup 1.0000x reference)
"""Trainium2 Bass kernel for nn_BlockWithCompression (dense transformer block).

Sharding: 8 cores = 4 batches x 2 query-parities. Core (b, par) computes the
full block output for batch b at query token blocks {2s+par : s=0..7} (128
tokens each). K/V are computed for the full sequence on every core (duplicated
across the pair); attention exploits causality: slot s attends to key blocks
[0, 2s+2), with the mask supplied as per-core input data so the instruction
stream is identical on all 8 cores (SPMD). No collectives.

Layouts: activations are feature-major ("xT": [E on partitions, tokens free])
so matmuls need no on-device transposes except the initial PE-transpose of x.
Scores are computed transposed ([key, query]); softmax denominators come from
a ones-column appended to V; normalization happens at PSUM-evict time.

Dtypes: attention path (LN1/Q/K/V/wei) in bf16; proj/FFN/LN2 matmuls in
float32r (1 cycle/row, ~1.5e-4 rel err). All accumulation in fp32 PSUM.
SBUF tiles share slots via lifetime-chained tags (hT->xTq2->ffT etc).
"""

import numpy as np

B, T, E, H = 4, 2048, 1024, 16
HS = E // H          # 64
FF = 4 * E           # 4096
P = 128
NQ = T // 2          # 1024 query tokens per core
NCORES = 8
MASK_NEG = -30000.0
SCALE = float(E) ** -0.5
EPS = 1e-5

_CACHE = {}


def _build_nc(debug_taps=False):
    from contextlib import ExitStack

    import concourse.tile as tile
    import concourse.mybir as mybir
    from concourse import bacc
    from concourse.masks import make_identity

    dt = mybir.dt
    AF = mybir.ActivationFunctionType
    ALU = mybir.AluOpType

    nc = bacc.Bacc("TRN2", target_bir_lowering=False, debug=False,
                   num_devices=NCORES)

    x_d = nc.dram_tensor("x", [T, E], dt.float32, kind="ExternalInput")
    xq_d = nc.dram_tensor("xq", [NQ, E], dt.float32, kind="ExternalInput")
    wq_d = nc.dram_tensor("wq", [E, E], dt.bfloat16, kind="ExternalInput")
    wk_d = nc.dram_tensor("wk", [E, E], dt.bfloat16, kind="ExternalInput")
    wv_d = nc.dram_tensor("wv", [E, E], dt.bfloat16, kind="ExternalInput")
    wp_d = nc.dram_tensor("wp", [E, E], dt.float32r, kind="ExternalInput")
    w1_d = nc.dram_tensor("w1", [E, FF], dt.float32r, kind="ExternalInput")
    w2_d = nc.dram_tensor("w2", [FF, E], dt.float32r, kind="ExternalInput")
    bq_d = nc.dram_tensor("bq", [E], dt.float32, kind="ExternalInput")
    bk_d = nc.dram_tensor("bk", [E], dt.float32, kind="ExternalInput")
    bv_d = nc.dram_tensor("bv", [1, E], dt.float32, kind="ExternalInput")
    bp_d = nc.dram_tensor("bp", [E], dt.float32, kind="ExternalInput")
    b1_d = nc.dram_tensor("b1", [FF], dt.float32, kind="ExternalInput")
    b2_d = nc.dram_tensor("b2", [E], dt.float32, kind="ExternalInput")
    mask_d = nc.dram_tensor("maskt", [16, P, P], dt.float32,
                            kind="ExternalInput")
    out_d = nc.dram_tensor("out", [E, NQ], dt.float32, kind="ExternalOutput")
    if debug_taps:
        dbg_hT = nc.dram_tensor("dbg_hT", [P, 8, T], dt.bfloat16,
                                kind="ExternalOutput")
        dbg_KT = nc.dram_tensor("dbg_KT", [P, 8, T], dt.bfloat16,
                                kind="ExternalOutput")
        dbg_QT = nc.dram_tensor("dbg_QT", [P, 8, NQ], dt.bfloat16,
                                kind="ExternalOutput")
        dbg_V = nc.dram_tensor("dbg_V", [P, 16, H * 65], dt.bfloat16,
                               kind="ExternalOutput")
        dbg_attnT = nc.dram_tensor("dbg_attnT", [P, 8, NQ], dt.float32,
                                   kind="ExternalOutput")
        dbg_yT = nc.dram_tensor("dbg_yT", [P, 8, NQ], dt.float32,
                                kind="ExternalOutput")
        dbg_h2T = nc.dram_tensor("dbg_h2T", [P, 8, NQ], dt.float32,
                                 kind="ExternalOutput")

    EC = E // P    # 8 feature chunks
    TC = T // P    # 16 token blocks

    with tile.TileContext(nc) as tc, ExitStack() as top:
        const = top.enter_context(tc.tile_pool(name="const", bufs=1))
        ident = const.tile([P, P], dt.float32)
        make_identity(nc, ident)
        ones_f = const.tile([P, 1], dt.float32)
        nc.vector.memset(ones_f[:], 1.0)
        ones_r = const.tile([P, 1], dt.float32r)
        nc.vector.tensor_copy(ones_r[:], ones_f[:])
        ones_b = const.tile([P, 1], dt.bfloat16)
        nc.vector.tensor_copy(ones_b[:], ones_f[:])

        persist = top.enter_context(tc.tile_pool(name="persist", bufs=1))

        def layernorm(src_t, dst_t, ntok, spool, sqpool, rowpool, bpool,
                      sq_dt, ones_t, bc_dt, lbl):
            """dst_t = layernorm(src_t) (no affine); dst may equal src.
            src_t: [P, EC, ntok] feature-major. Processes 512-token chunks:
            stats via ones-matmuls (partition reduction), then
            dst = src * rstd - mu * rstd with gpsimd-broadcast rows."""
            for t4 in range(ntok // 512):
                sl = slice(t4 * 512, (t4 + 1) * 512)
                sums = spool.tile([1, 512], dt.float32,
                                  name=f"sums_{lbl}_{t4}", tag="stat_sums")
                sqs = spool.tile([1, 512], dt.float32,
                                 name=f"sqs_{lbl}_{t4}", tag="stat_sqs")
                for ec in range(EC):
                    nc.tensor.matmul(sums[:], ones_t[:], src_t[:, ec, sl],
                                     start=(ec == 0), stop=(ec == EC - 1))
                for ec in range(EC):
                    xsq = sqpool.tile([P, 512], sq_dt,
                                      name=f"xsq_{lbl}_{t4}_{ec}",
                                      tag="stat_xsq")
                    nc.scalar.activation(xsq[:], src_t[:, ec, sl], AF.Square)
                    nc.tensor.matmul(sqs[:], ones_t[:], xsq[:],
                                     start=(ec == 0), stop=(ec == EC - 1))
                mu = rowpool.tile([1, 512], dt.float32,
                                  name=f"mu_{lbl}_{t4}", tag="stat_mu")
                nc.vector.tensor_scalar_mul(mu[:], sums[:], 1.0 / E)
                musq = rowpool.tile([1, 512], dt.float32,
                                    name=f"musq_{lbl}_{t4}", tag="stat_musq")
                nc.vector.tensor_mul(musq[:], mu[:], mu[:])
                var = rowpool.tile([1, 512], dt.float32,
                                   name=f"var_{lbl}_{t4}", tag="stat_var")
                nc.vector.scalar_tensor_tensor(
                    var[:], sqs[:], 1.0 / E, musq[:],
                    op0=ALU.mult, op1=ALU.subtract)
                nc.vector.tensor_scalar_add(var[:], var[:], EPS)
                rec = rowpool.tile([1, 512], dt.float32,
                                   name=f"rec_{lbl}_{t4}", tag="stat_rec")
                nc.vector.reciprocal(rec[:], var[:])
                rstd = rowpool.tile([1, 512], dt.float32,
                                    name=f"rstd_{lbl}_{t4}", tag="stat_rstd")
                nc.scalar.activation(rstd[:], rec[:], AF.Sqrt)
                m2 = rowpool.tile([1, 512], dt.float32,
                                  name=f"m2_{lbl}_{t4}", tag="stat_m2")
                nc.vector.tensor_mul(m2[:], mu[:], rstd[:])
                m2b = bpool.tile([P, 512], bc_dt,
                                 name=f"m2b_{lbl}_{t4}", tag="ln_m2b")
                rstdb = bpool.tile([P, 512], bc_dt,
                                   name=f"rstdb_{lbl}_{t4}", tag="ln_rstdb")
                nc.gpsimd.partition_broadcast(m2b[:], m2[:])
                nc.gpsimd.partition_broadcast(rstdb[:], rstd[:])
                for ec in range(EC):
                    nc.vector.tensor_mul(dst_t[:, ec, sl], src_t[:, ec, sl],
                                         rstdb[:])
                    nc.vector.tensor_sub(dst_t[:, ec, sl], dst_t[:, ec, sl],
                                         m2b[:])

        def transpose_in(dram_ap, nrows, dst_t, xpool, tps, label):
            """DMA token-major [nrows, E]; PE-transpose into dst_t
            [P, EC, nrows]."""
            for tcb in range(nrows // P):
                xtok = xpool.tile([P, E], dt.float32,
                                  name=f"xtok_{label}_{tcb}", tag="xtok")
                nc.sync.dma_start(xtok[:], dram_ap[tcb * P:(tcb + 1) * P, :])
                for ec in range(EC):
                    tp = tps.tile([P, P], dt.float32,
                                  name=f"tp_{label}_{tcb}_{ec}", tag="tp")
                    nc.tensor.transpose(tp[:], xtok[:, ec * P:(ec + 1) * P],
                                        ident[:])
                    nc.vector.tensor_copy(
                        dst_t[:, ec, tcb * P:(tcb + 1) * P], tp[:])

        # ============ PHASE A: x -> xT -> LN1 (in place) -> hT ============
        # slot chain "sA": hT(A-B) -> xTq2(D) -> ffT(E)  [32 KB/part]
        hT = persist.tile([P, EC, T], dt.bfloat16, name="hT", tag="sA")
        with ExitStack() as ph:
            pa = ph.enter_context(tc.tile_pool(name="pa", bufs=1))
            xpool = ph.enter_context(tc.tile_pool(name="pa_x", bufs=3))
            tps = ph.enter_context(tc.tile_pool(name="pa_tp", bufs=3,
                                                space="PSUM"))
            spool = ph.enter_context(tc.tile_pool(name="pa_st", bufs=1,
                                                  space="PSUM"))
            sqpool = ph.enter_context(tc.tile_pool(name="pa_sq", bufs=3))
            rowpool = ph.enter_context(tc.tile_pool(name="pa_row", bufs=1))
            bpool = ph.enter_context(tc.tile_pool(name="pa_b", bufs=1))

            transpose_in(x_d.ap(), T, hT, xpool, tps, "a")
            layernorm(hT, hT, T, spool, sqpool, rowpool, bpool,
                      dt.bfloat16, ones_b, dt.float32, "a")

        if debug_taps:
            nc.sync.dma_start(dbg_hT.ap(), hT[:])

        # ============ PHASE B: QKV projections ============
        # "sB": KT(B-C) -> h2T(D-E); "sC": V(B-C) -> yT(D-E)
        # "sD": hTq(B) -> attnT(C-D) -> oacc(E); "sE": QT(B-C)
        KT = persist.tile([P, EC, T], dt.bfloat16, name="KT", tag="sB")
        QT = persist.tile([P, EC, NQ], dt.bfloat16, name="QT", tag="sE")
        V = persist.tile([P, TC, H * 65], dt.bfloat16, name="V", tag="sC")
        with ExitStack() as ph:
            wpool = ph.enter_context(tc.tile_pool(name="pb_w", bufs=2))
            bps = ph.enter_context(tc.tile_pool(name="pb_ps", bufs=3,
                                                space="PSUM"))
            biasp = ph.enter_context(tc.tile_pool(name="pb_bias", bufs=1))
            pbv = ph.enter_context(tc.tile_pool(name="pb_bv", bufs=1))

            # --- Q section: xq -> xTq -> LN (in place) -> hTq -> QT ---
            with ExitStack() as qh:
                pq = qh.enter_context(tc.tile_pool(name="pq", bufs=1))
                xpool = qh.enter_context(tc.tile_pool(name="pq_x", bufs=3))
                tps = qh.enter_context(tc.tile_pool(name="pq_tp", bufs=3,
                                                    space="PSUM"))
                spool = qh.enter_context(tc.tile_pool(name="pq_st", bufs=1,
                                                      space="PSUM"))
                sqpool = qh.enter_context(tc.tile_pool(name="pq_sq", bufs=3))
                rowpool = qh.enter_context(tc.tile_pool(name="pq_row",
                                                        bufs=1))
                bpool = qh.enter_context(tc.tile_pool(name="pq_b", bufs=1))

                hTq = persist.tile([P, EC, NQ], dt.bfloat16, name="hTq",
                                   tag="sD")
                transpose_in(xq_d.ap(), NQ, hTq, xpool, tps, "bq")
                layernorm(hTq, hTq, NQ, spool, sqpool, rowpool, bpool,
                          dt.bfloat16, ones_b, dt.float32, "bq")

                bq_sb = biasp.tile([P, EC], dt.float32, name="bq_sb")
                nc.sync.dma_start(bq_sb[:],
                                  bq_d.ap().rearrange("(c p) -> p c", p=P))
                for half in range(2):
                    wt = wpool.tile([P, EC, E // 2], dt.bfloat16,
                                    name=f"wt_q_{half}", tag="w")
                    src = wq_d.ap().rearrange("(c p) n -> p c n", p=P)
                    nc.sync.dma_start(
                        wt[:], src[:, :, half * 512:(half + 1) * 512])
                    for eo4 in range(4):
                        eo = half * 4 + eo4
                        for qc in range(NQ // 512):
                            sl = slice(qc * 512, (qc + 1) * 512)
                            pp = bps.tile([P, 512], dt.float32,
                                          name=f"pp_q_{eo}_{qc}",
                                          tag="projps")
                            for ei in range(EC):
                                nc.tensor.matmul(
                                    pp[:], wt[:, ei, eo4 * P:(eo4 + 1) * P],
                                    hTq[:, ei, sl],
                                    start=(ei == 0), stop=(ei == EC - 1))
                            nc.scalar.activation(QT[:, eo, sl], pp[:],
                                                 AF.Identity,
                                                 bias=bq_sb[:, eo:eo + 1])

            # --- K section ---
            bk_sb = biasp.tile([P, EC], dt.float32, name="bk_sb")
            nc.sync.dma_start(bk_sb[:],
                              bk_d.ap().rearrange("(c p) -> p c", p=P))
            for half in range(2):
                wt = wpool.tile([P, EC, E // 2], dt.bfloat16,
                                name=f"wt_k_{half}", tag="w")
                src = wk_d.ap().rearrange("(c p) n -> p c n", p=P)
                nc.sync.dma_start(wt[:],
                                  src[:, :, half * 512:(half + 1) * 512])
                for eo4 in range(4):
                    eo = half * 4 + eo4
                    for qc in range(T // 512):
                        sl = slice(qc * 512, (qc + 1) * 512)
                        pp = bps.tile([P, 512], dt.float32,
                                      name=f"pp_k_{eo}_{qc}", tag="projps")
                        for ei in range(EC):
                            nc.tensor.matmul(
                                pp[:], wt[:, ei, eo4 * P:(eo4 + 1) * P],
                                hT[:, ei, sl],
                                start=(ei == 0), stop=(ei == EC - 1))
                        nc.scalar.activation(KT[:, eo, sl], pp[:],
                                             AF.Identity,
                                             bias=bk_sb[:, eo:eo + 1])

            # --- V section: token-major with ones column per head ---
            bv_row = biasp.tile([1, E], dt.float32, name="bv_row")
            nc.sync.dma_start(bv_row[:], bv_d.ap())
            bvb = pbv.tile([P, E], dt.float32, name="bvb")
            nc.gpsimd.partition_broadcast(bvb[:], bv_row[:])
            nc.vector.memset(V[:, :, 64::65], 1.0)
            for half in range(2):
                wt = wpool.tile([P, EC, E // 2], dt.bfloat16,
                                name=f"wt_v_{half}", tag="w")
                src = wv_d.ap().rearrange("(c p) n -> p c n", p=P)
                nc.sync.dma_start(wt[:],
                                  src[:, :, half * 512:(half + 1) * 512])
                h0 = half * 8
                for tcb in range(TC):
                    tb = slice(tcb * P, (tcb + 1) * P)
                    pp = bps.tile([P, 512], dt.float32,
                                  name=f"ppv_{half}_{tcb}", tag="projps")
                    for ei in range(EC):
                        nc.tensor.matmul(pp[:], hT[:, ei, tb], wt[:, ei, :],
                                         start=(ei == 0), stop=(ei == EC - 1))
                    dst = V[:, tcb, :].rearrange(
                        "p (h w) -> p h w", w=65)[:, h0:h0 + 8, 0:64]
                    nc.vector.tensor_add(
                        dst, pp[:], bvb[:, half * 512:(half + 1) * 512])

        if debug_taps:
            nc.sync.dma_start(dbg_KT.ap(), KT[:])
            nc.sync.dma_start(dbg_QT.ap(), QT[:])
            nc.sync.dma_start(dbg_V.ap(), V[:])

        # ============ PHASE C: attention ============
        attnT = persist.tile([P, EC, NQ], dt.float32r, name="attnT", tag="sD")
        with ExitStack() as ph:
            pc = ph.enter_context(tc.tile_pool(name="pc", bufs=1))
            score_ps = ph.enter_context(tc.tile_pool(name="pc_sc", bufs=2,
                                                     space="PSUM"))
            attn_ps = ph.enter_context(tc.tile_pool(name="pc_at", bufs=2,
                                                    space="PSUM"))
            weip = ph.enter_context(tc.tile_pool(name="pc_wei", bufs=4))
            rowp = ph.enter_context(tc.tile_pool(name="pc_row", bufs=2))

            masks_sb = pc.tile([P, 16, P], dt.float32, name="masks_sb")
            nc.sync.dma_start(masks_sb[:],
                              mask_d.ap().rearrange("k p q -> p k q"))

            for h in range(H):
                til = h // 2
                r0 = (h % 2) * 64
                aps = attn_ps.tile([65, NQ], dt.float32,
                                   name=f"aps_{h}", tag="aps")
                for kc in range(TC):
                    n0 = (kc // 2) * P
                    NW = NQ - n0
                    sps = score_ps.tile([P, NQ], dt.float32,
                                        name=f"sps_{h}_{kc}", tag="sc")
                    nsp = (NW + 511) // 512
                    for j in range(nsp):
                        a = n0 + j * 512
                        b = min(NQ, a + 512)
                        nc.tensor.matmul(
                            sps[:, a - n0:b - n0],
                            KT[r0:r0 + 64, til, kc * P:(kc + 1) * P],
                            QT[r0:r0 + 64, til, a:b],
                            start=True, stop=True,
                            tile_position=(r0, 0))
                    nc.vector.tensor_add(sps[:, 0:P], sps[:, 0:P],
                                         masks_sb[:, kc, :])
                    wei = weip.tile([P, NW], dt.bfloat16,
                                    name=f"wei_{h}_{kc}", tag="wei")
                    nc.scalar.activation(wei[:], sps[:, 0:NW], AF.Exp,
                                         scale=SCALE)
                    for j in range(nsp):
                        a = n0 + j * 512
                        b = min(NQ, a + 512)
                        nc.tensor.matmul(
                            aps[:, a:b],
                            V[:, kc, h * 65:(h + 1) * 65],
                            wei[:, a - n0:b - n0],
                            start=(kc == 0), stop=(kc == TC - 1),
                            skip_group_check=True)
                rrow = rowp.tile([1, NQ], dt.float32,
                                 name=f"rrow_{h}", tag="rrow")
                nc.vector.reciprocal(rrow[:], aps[64:65, :])
                rb = rowp.tile([64, NQ], dt.float32, name=f"rb_{h}", tag="rb")
                nc.gpsimd.partition_broadcast(rb[:], rrow[:])
                nc.vector.tensor_mul(attnT[r0:r0 + 64, til, :],
                                     aps[0:64, :], rb[:])

        if debug_taps:
            nc.sync.dma_start(dbg_attnT.ap(), attnT[:].bitcast(dt.float32))

        # ============ PHASE D: proj + residual + LN2 ============
        yT = persist.tile([P, EC, NQ], dt.float32r, name="yT", tag="sC")
        h2T = persist.tile([P, EC, NQ], dt.float32r, name="h2T", tag="sB")
        with ExitStack() as ph:
            pd = ph.enter_context(tc.tile_pool(name="pd", bufs=1))
            wpool = ph.enter_context(tc.tile_pool(name="pd_w", bufs=2))
            dps = ph.enter_context(tc.tile_pool(name="pd_ps", bufs=3,
                                                space="PSUM"))
            spool = ph.enter_context(tc.tile_pool(name="pd_st", bufs=1,
                                                  space="PSUM"))
            sqpool = ph.enter_context(tc.tile_pool(name="pd_sq", bufs=3))
            rowpool = ph.enter_context(tc.tile_pool(name="pd_row", bufs=1))
            bpool = ph.enter_context(tc.tile_pool(name="pd_b", bufs=1))
            xpool = ph.enter_context(tc.tile_pool(name="pd_x", bufs=3))
            tps = ph.enter_context(tc.tile_pool(name="pd_tp", bufs=3,
                                                space="PSUM"))
            biasp = ph.enter_context(tc.tile_pool(name="pd_bias", bufs=1))

            xTq2 = persist.tile([P, EC, NQ], dt.float32r, name="xTq2",
                                tag="sA")
            transpose_in(xq_d.ap(), NQ, xTq2, xpool, tps, "d")

            bp_sb = biasp.tile([P, EC], dt.float32, name="bp_sb")
            nc.sync.dma_start(bp_sb[:],
                              bp_d.ap().rearrange("(c p) -> p c", p=P))

            for quarter in range(4):
                wt = wpool.tile([P, EC, E // 4], dt.float32r,
                                name=f"wt_p_{quarter}", tag="w")
                src = wp_d.ap().rearrange("(c p) n -> p c n", p=P)
                nc.sync.dma_start(
                    wt[:], src[:, :, quarter * 256:(quarter + 1) * 256])
                for eo2 in range(2):
                    eo = quarter * 2 + eo2
                    for qc in range(2):
                        sl = slice(qc * 512, (qc + 1) * 512)
                        pp = dps.tile([P, 512], dt.float32,
                                      name=f"ppp_{eo}_{qc}", tag="projps")
                        for ei in range(EC):
                            nc.tensor.matmul(
                                pp[:], wt[:, ei, eo2 * P:(eo2 + 1) * P],
                                attnT[:, ei, sl],
                                start=(ei == 0), stop=(ei == EC - 1))
                        nc.vector.scalar_tensor_tensor(
                            yT[:, eo, sl], pp[:], bp_sb[:, eo:eo + 1],
                            xTq2[:, eo, sl], op0=ALU.add, op1=ALU.add)

            layernorm(yT, h2T, NQ, spool, sqpool, rowpool, bpool,
                      dt.float32r, ones_r, dt.float32, "d")

        if debug_taps:
            nc.sync.dma_start(dbg_yT.ap(), yT[:].bitcast(dt.float32))
            nc.sync.dma_start(dbg_h2T.ap(), h2T[:].bitcast(dt.float32))

        # ============ PHASE E: FFN + residual -> out ============
        with ExitStack() as ph:
            w1pool = ph.enter_context(tc.tile_pool(name="pe_w1", bufs=2))
            w2pool = ph.enter_context(tc.tile_pool(name="pe_w2", bufs=3))
            e1ps = ph.enter_context(tc.tile_pool(name="pe_ps1", bufs=3,
                                                 space="PSUM"))
            e2ps = ph.enter_context(tc.tile_pool(name="pe_ps2", bufs=3,
                                                 space="PSUM"))
            tmpp = ph.enter_context(tc.tile_pool(name="pe_tmp", bufs=3))
            biasp = ph.enter_context(tc.tile_pool(name="pe_bias", bufs=1))

            b1_sb = biasp.tile([P, FF // P], dt.float32, name="b1_sb")
            b2_sb = biasp.tile([P, EC], dt.float32, name="b2_sb")
            nc.sync.dma_start(b1_sb[:],
                              b1_d.ap().rearrange("(c p) -> p c", p=P))
            nc.sync.dma_start(b2_sb[:],
                              b2_d.ap().rearrange("(c p) -> p c", p=P))

            oacc = persist.tile([P, EC, NQ], dt.float32, name="oacc",
                                tag="sD")
            w1_src = w1_d.ap().rearrange("(c p) n -> p c n", p=P)
            w2_src = w2_d.ap().rearrange("(q g p) n -> p q g n", g=8, p=P)

            for fq in range(4):
                ffT = persist.tile([P, 8, NQ], dt.float32r,
                                   name=f"ffT_{fq}", tag="sA")
                for half in range(2):
                    w1t = w1pool.tile([P, EC, 512], dt.float32r,
                                      name=f"w1t_{fq}_{half}", tag="w1")
                    nc.sync.dma_start(
                        w1t[:], w1_src[:, :, fq * 1024 + half * 512:
                                       fq * 1024 + (half + 1) * 512])
                    for fg4 in range(4):
                        fg = half * 4 + fg4
                        for qc in range(2):
                            sl = slice(qc * 512, (qc + 1) * 512)
                            pp = e1ps.tile([P, 512], dt.float32,
                                           name=f"pp1_{fq}_{fg}_{qc}",
                                           tag="ff1ps")
                            for ei in range(EC):
                                nc.tensor.matmul(
                                    pp[:], w1t[:, ei, fg4 * P:(fg4 + 1) * P],
                                    h2T[:, ei, sl],
                                    start=(ei == 0), stop=(ei == EC - 1))
                            nc.scalar.activation(
                                ffT[:, fg, sl], pp[:], AF.Relu,
                                bias=b1_sb[:, fq * 8 + fg:fq * 8 + fg + 1])
                for eo in range(EC):
                    w2t = w2pool.tile([P, 8, P], dt.float32r,
                                      name=f"w2t_{fq}_{eo}", tag="w2")
                    nc.sync.dma_start(
                        w2t[:], w2_src[:, fq, :, eo * P:(eo + 1) * P])
                    for qc in range(2):
                        sl = slice(qc * 512, (qc + 1) * 512)
                        pp = e2ps.tile([P, 512], dt.float32,
                                       name=f"pp2_{fq}_{eo}_{qc}",
                                       tag="ff2ps")
                        for fg in range(8):
                            nc.tensor.matmul(pp[:], w2t[:, fg, :],
                                             ffT[:, fg, sl],
                                             start=(fg == 0), stop=(fg == 7))
                        if fq == 0:
                            nc.vector.tensor_copy(oacc[:, eo, sl], pp[:])
                        elif fq < 3:
                            nc.vector.tensor_add(oacc[:, eo, sl],
                                                 oacc[:, eo, sl], pp[:])
                        else:
                            tmp = tmpp.tile([P, 512], dt.float32,
                                            name=f"tmpo_{eo}_{qc}",
                                            tag="tmpo")
                            nc.vector.scalar_tensor_tensor(
                                tmp[:], pp[:], b2_sb[:, eo:eo + 1],
                                oacc[:, eo, sl], op0=ALU.add, op1=ALU.add)
                            nc.vector.tensor_add(oacc[:, eo, sl], tmp[:],
                                                 yT[:, eo, sl])
                            if qc == 1:
                                nc.sync.dma_start(
                                    out_d.ap()[eo * P:(eo + 1) * P, :],
                                    oacc[:, eo, :])

    nc.compile()
    return nc


def _make_masks(parity: int) -> np.ndarray:
    """Additive masks for scoresT chunks [16, P(key), P(query)]."""
    k = np.arange(P)[:, None]
    q = np.arange(P)[None, :]
    tril = np.where(k <= q, 0.0, MASK_NEG).astype(np.float32)
    zeros = np.zeros((P, P), np.float32)
    full = np.full((P, P), MASK_NEG, np.float32)
    m = np.empty((16, P, P), np.float32)
    for s in range(8):
        if parity == 0:
            m[2 * s] = tril
            m[2 * s + 1] = full
        else:
            m[2 * s] = zeros
            m[2 * s + 1] = tril
    return m


def kernel(**inputs) -> np.ndarray:
    import ml_dtypes

    x = np.asarray(inputs["x"], np.float32)
    g1 = np.asarray(inputs["g1"], np.float32)
    be1 = np.asarray(inputs["be1"], np.float32)
    g2 = np.asarray(inputs["g2"], np.float32)
    be2 = np.asarray(inputs["be2"], np.float32)
    Wq = np.asarray(inputs["Wq"], np.float32)
    Wk = np.asarray(inputs["Wk"], np.float32)
    Wv = np.asarray(inputs["Wv"], np.float32)
    Wp = np.asarray(inputs["Wp"], np.float32)
    bp = np.asarray(inputs["bp"], np.float32)
    W1 = np.asarray(inputs["W1"], np.float32)
    b1 = np.asarray(inputs["b1"], np.float32)
    W2 = np.asarray(inputs["W2"], np.float32)
    b2 = np.asarray(inputs["b2"], np.float32)

    # fold layernorm affine params into the adjacent projections
    bf16 = ml_dtypes.bfloat16
    wq = (g1[:, None] * Wq).astype(bf16)
    wk = (g1[:, None] * Wk).astype(bf16)
    wv = (g1[:, None] * Wv).astype(bf16)
    bq = be1 @ Wq
    bk = be1 @ Wk
    bv = (be1 @ Wv)[None, :]
    w1 = g2[:, None] * W1
    b1f = b1 + be2 @ W1

    if "nc" not in _CACHE:
        _CACHE["nc"] = _build_nc()
    nc = _CACHE["nc"]

    masks = [_make_masks(0), _make_masks(1)]
    shared = dict(wq=wq, wk=wk, wv=wv, wp=Wp, w1=w1, w2=W2,
                  bq=bq, bk=bk, bv=bv, bp=bp, b1=b1f, b2=b2)
    in_maps = []
    for core in range(NCORES):
        b = core // 2
        par = core % 2
        xb = x[b]
        xq = xb.reshape(16, P, E)[par::2].reshape(NQ, E).copy()
        in_maps.append(dict(x=xb, xq=xq, maskt=masks[par], **shared))

    from concourse.bass_utils import run_bass_kernel_spmd
    res = run_bass_kernel_spmd(nc, in_maps, core_ids=list(range(NCORES)))

    out = np.empty((B, T, E), np.float32)
    for core in range(NCORES):
        b = core // 2
        par = core % 2
        o = res.results[core]["out"]          # [E, NQ]
        blocks = np.ascontiguousarray(o.T).reshape(8, P, E)
        out[b].reshape(16, P, E)[par::2] = blocks
    return out
